# revision 1
# baseline (speedup 1.0000x reference)
"""Trainium2 Bass kernel for nn_AttentionLayer (conv1d -> linear attention -> gelu + residual).

Full inputs:  x [8, 256, 4096] f32, conv_w [512, 256, 3] f32, conv_b [512] f32
Full output:  [8, 256, 4096] f32

Sharding: pure data-parallel over batch B=8 -> 8 NeuronCores, one batch each.
No collectives needed.

Per-core math (C=256, N=4096, one batch):
  y    = conv1d(x, w, pad=1) + b          # [2C, N]
  q    = phi(y[:C]),  k = phi(y[C:])      # phi = elu+1 = max(y+1, exp(min(y,0)))
  v    = x^T                              # [N, C]
  kv   = sum_n phi(k)[n,:] (x) v[n,:]     # [C, C]
  out  = gelu(q @ kv) + x                 # [C, N]

Layout trick: the conv contraction (over input channels ci) lets us produce
q in [c, n] layout (w^T as stationary operand) AND k in [n, c] layout
(x as stationary operand) with zero transposes. v^T (= x^T) is shipped
pre-transposed/pre-cast from the host, as are bf16 copies of x and w.

Matmuls run in bf16 (f32 PSUM accumulate): bf16 gets pipelined LDWEIGHTS
(f32/f32r matmuls serialize a ~107ns self-weight-load per matmul).
phi is 3 ops via the fused scalar_tensor_tensor: min (DVE) ->
exp (ACT, one table per phase) -> (y+1) max e (DVE). The conv bias for
the k half enters as a rank-1 start matmul (ones^T @ b_k); for the q half
it rides the DVE ops' per-partition scalar operand. Residual add uses an
f32 copy of x on GpSimd (the only engine with slack).
"""

import ml_dtypes
import numpy as np

import concourse.bass as bass
import concourse.mybir as mybir
import concourse.tile as tile
from concourse import bacc
from concourse.bass_utils import run_bass_kernel_spmd

F32 = mybir.dt.float32
BF16 = mybir.dt.bfloat16
AF = mybir.ActivationFunctionType
ALU = mybir.AluOpType

B, C, N = 8, 256, 4096
NCORES = 8
CT = C // 128        # 2 c-tiles (partition groups) per 256-channel dim
NJ = N // 512        # 8 column chunks of 512
NT = N // 128        # 32 n-tiles of 128
NP = N + 2           # x padded with one zero column on each side

BF = ml_dtypes.bfloat16


def _build_nc():
    nc = bacc.Bacc("TRN2", target_bir_lowering=False, debug=False, num_devices=NCORES)

    xb_d = nc.declare_dram_parameter("xb", [C, NP], BF16, isOutput=False)
    xt_d = nc.declare_dram_parameter("xt", [N, C], BF16, isOutput=False)
    wt_d = nc.declare_dram_parameter("wt", [3, CT, 128, 512], BF16, isOutput=False)
    bq_d = nc.declare_dram_parameter("bq", [CT, 128, 1], F32, isOutput=False)
    bq1_d = nc.declare_dram_parameter("bq1", [CT, 128, 1], F32, isOutput=False)
    bk_d = nc.declare_dram_parameter("bk", [2, 256], BF16, isOutput=False)
    out_d = nc.declare_dram_parameter("out", [C, N], F32, isOutput=True)

    with tile.TileContext(nc) as tc:
        with (
            tc.tile_pool(name="persist", bufs=1) as per,
            tc.tile_pool(name="tmp", bufs=6) as tmp,
            tc.tile_pool(name="psum", bufs=6, space="PSUM") as ps,
            tc.tile_pool(name="psum2", bufs=2, space="PSUM") as ps2,
        ):
            # ---- constants / weights -------------------------------------
            ones = per.tile([1, 128], BF16, tag="ones")
            nc.sync.dma_start(out=ones, in_=bk_d[0:1, 0:128])
            bk_sb = per.tile([1, 256], BF16, tag="bk")
            nc.sync.dma_start(out=bk_sb, in_=bk_d[1:2, :])
            bq_sb = per.tile([128, CT, 1], F32, tag="bq")
            bq1_sb = per.tile([128, CT, 1], F32, tag="bq1")
            for ct in range(CT):
                nc.sync.dma_start(out=bq_sb[:, ct, :], in_=bq_d[ct, :, :])
                nc.sync.dma_start(out=bq1_sb[:, ct, :], in_=bq1_d[ct, :, :])

            wt_sb = [[per.tile([128, 512], BF16, tag=f"wt{t}{ci}", name=f"wt{t}{ci}")
                      for ci in range(CT)] for t in range(3)]
            for t in range(3):
                for ci in range(CT):
                    nc.sync.dma_start(out=wt_sb[t][ci][:, 256:512],
                                      in_=wt_d[t, ci, :, 256:512])

            # bf16 x chunks for the conv matmuls; chunk 0 first (head)
            xs = [[per.tile([128, 514], BF16, tag=f"x{ci}{j}", name=f"x{ci}{j}")
                   for j in range(NJ)] for ci in range(CT)]
            for j in range(NJ):
                for ci in range(CT):
                    nc.sync.dma_start(
                        out=xs[ci][j],
                        in_=xb_d[ci * 128:(ci + 1) * 128, j * 512:j * 512 + 514],
                    )
            for t in range(3):
                for ci in range(CT):
                    nc.sync.dma_start(out=wt_sb[t][ci][:, 0:256],
                                      in_=wt_d[t, ci, :, 0:256])

            # v^T straight from DRAM (host pre-transposed bf16)
            vT = per.tile([128, NT, 256], BF16, tag="vT")
            nc.sync.dma_start(
                out=vT, in_=xt_d.rearrange("(i p) d -> p i d", p=128))

            # ---- persistent intermediates --------------------------------
            kT = per.tile([128, NT, 256], BF16, tag="kT")    # phi(k) in [n, c]
            qphi = [per.tile([128, N], BF16, tag=f"qphi{ct}", name=f"qphi{ct}")
                    for ct in range(CT)]
            kv_sb = per.tile([128, CT, 256], BF16, tag="kv")  # kv in [c, d]

            # ---- phase NT: k^T (conv in transposed layout) ---------------
            for i in range(NT):
                j, off = i // 4, (i % 4) * 128
                kt_ps = ps.tile([128, 512], F32, tag="bank", name="kt_ps")
                # bias row: ones^T @ bk broadcasts conv_b[k-half] over rows
                kt_ps = kt_ps[:, 0:256]
                nc.tensor.matmul(kt_ps, ones, bk_sb, start=True, stop=False)
                for ci in range(CT):
                    for t in range(3):
                        nc.tensor.matmul(
                            kt_ps,
                            xs[ci][j][:, off + t:off + t + 128],
                            wt_sb[t][ci][:, 256:512],
                            start=False,
                            stop=(ci == CT - 1 and t == 2),
                        )
                # phi: kT = max(y+1, exp(min(y, 0)))
                tmin = tmp.tile([128, 256], F32, tag="ntmin")
                nc.vector.tensor_scalar(tmin, kt_ps, 0.0, None, ALU.min)
                e = tmp.tile([128, 256], F32, tag="nte")
                nc.scalar.activation(e, tmin, AF.Exp)
                nc.vector.scalar_tensor_tensor(
                    kT[:, i, :], kt_ps, 1.0, e, ALU.add, ALU.max)

            # ---- phase Q: conv q in [c, n] layout ------------------------
            for ct in range(CT):
                for j in range(NJ):
                    q_ps = ps.tile([128, 512], F32, tag="bank", name="q_ps")
                    first = True
                    for ci in range(CT):
                        for t in range(3):
                            nc.tensor.matmul(
                                q_ps,
                                wt_sb[t][ci][:, ct * 128:(ct + 1) * 128],
                                xs[ci][j][:, t:t + 512],
                                start=first,
                                stop=(ci == CT - 1 and t == 2),
                            )
                            first = False
                    # phi with per-partition conv bias folded in:
                    #   min(y+b, 0) then (y + (b+1)) max exp(...)
                    tmin = tmp.tile([128, 512], F32, tag="qtmin")
                    nc.vector.tensor_scalar(
                        tmin, q_ps, bq_sb[:, ct, :], 0.0, ALU.add, ALU.min)
                    e = tmp.tile([128, 512], F32, tag="qte")
                    nc.scalar.activation(e, tmin, AF.Exp)
                    nc.vector.scalar_tensor_tensor(
                        qphi[ct][:, j * 512:(j + 1) * 512],
                        q_ps, bq1_sb[:, ct, :], e, ALU.add, ALU.max)

            # ---- phase KV: kv[c, d] = sum_n k^T[n, c] v^T[n, d] ----------
            for ch in range(CT):
                kv_ps = ps2.tile([128, 256], F32, tag="kvp", name="kv_ps")
                for i in range(NT):
                    nc.tensor.matmul(
                        kv_ps,
                        kT[:, i, ch * 128:(ch + 1) * 128],
                        vT[:, i, :],
                        start=(i == 0),
                        stop=(i == NT - 1),
                    )
                nc.scalar.copy(kv_sb[:, ch, :], kv_ps)

            # ---- phase OUT: out[d, n] = gelu(sum_c kv[c, d] q[c, n]) + x -
            for dt in range(CT):
                for j in range(NJ):
                    o_ps = ps.tile([128, 512], F32, tag="bank", name="o_ps")
                    for ch in range(CT):
                        nc.tensor.matmul(
                            o_ps,
                            kv_sb[:, ch, dt * 128:(dt + 1) * 128],
                            qphi[ch][:, j * 512:(j + 1) * 512],
                            start=(ch == 0),
                            stop=(ch == CT - 1),
                        )
                    g = tmp.tile([128, 512], F32, tag="og")
                    nc.scalar.activation(g, o_ps, AF.Gelu)
                    o = tmp.tile([128, 512], F32, tag="oo")
                    nc.gpsimd.tensor_add(o, g, xs[dt][j][:, 1:513])
                    nc.sync.dma_start(
                        out=out_d[dt * 128:(dt + 1) * 128, j * 512:(j + 1) * 512],
                        in_=o,
                    )

    nc.compile()
    return nc


_NC_CACHE = None


def _get_nc():
    global _NC_CACHE
    if _NC_CACHE is None:
        _NC_CACHE = _build_nc()
    return _NC_CACHE


def _prep(x, conv_w, conv_b):
    x = np.asarray(x, dtype=np.float32)
    conv_w = np.asarray(conv_w, dtype=np.float32)
    conv_b = np.asarray(conv_b, dtype=np.float32)
    xp = np.zeros((B, C, NP), dtype=BF)
    xp[:, :, 1:N + 1] = x.astype(BF)
    xt = np.ascontiguousarray(x.transpose(0, 2, 1)).astype(BF)   # [B, N, C]
    # wt[t, ci_tile, ci, co] = conv_w[co, ci_tile*128 + ci, t]
    wt = np.ascontiguousarray(
        conv_w.transpose(2, 1, 0).reshape(3, CT, 128, 2 * C)).astype(BF)
    bq = np.ascontiguousarray(conv_b[:C].reshape(CT, 128, 1))
    bq1 = np.ascontiguousarray(bq + 1.0)
    bk = np.ones((2, C), dtype=np.float32)
    bk[1, :] = conv_b[C:]
    bk = np.ascontiguousarray(bk).astype(BF)
    return xp, xt, wt, bq, bq1, bk


def make_in_maps(x, conv_w, conv_b):
    xp, xt, wt, bq, bq1, bk = _prep(x, conv_w, conv_b)
    return [
        {"xb": xp[b], "xt": xt[b], "wt": wt,
         "bq": bq, "bq1": bq1, "bk": bk}
        for b in range(B)
    ]


def kernel(x: np.ndarray, conv_w: np.ndarray, conv_b: np.ndarray) -> np.ndarray:
    nc = _get_nc()
    in_maps = make_in_maps(x, conv_w, conv_b)
    res = run_bass_kernel_spmd(nc, in_maps, core_ids=list(range(NCORES)))
    return np.stack([res.results[b]["out"] for b in range(B)], axis=0)



# revision 14
# speedup vs baseline: 1.2488x; 1.2488x over previous
"""Trainium2 Bass kernel for nn_AttentionLayer (conv1d -> linear attention -> gelu + residual).

Full inputs:  x [8, 256, 4096] f32, conv_w [512, 256, 3] f32, conv_b [512] f32
Full output:  [8, 256, 4096] f32

Sharding: pure data-parallel over batch B=8 -> 8 NeuronCores, one batch each.
No collectives needed.

Per-core math (C=256, N=4096, one batch):
  y    = conv1d(x, w, pad=1) + b          # [2C, N]
  q    = phi(y[:C]),  k = phi(y[C:])      # phi = elu+1
  v    = x^T                              # [N, C]
  kv   = sum_n phi(k)[n,:] (x) v[n,:]     # [C, C]
  out  = gelu(q @ kv) + x                 # [C, N]

Layout trick: the conv contraction (over input channels ci) produces
q in [c, n] layout (w^T stationary) AND k in [n, c] layout (x stationary)
with zero transposes; all other operands are host-prepped into layouts
that give one large contiguous-per-partition DMA per tensor (10 input
DMAs total — the HWDGE issue path serializes at ~650ns/DMA, so DMA
count is a first-order cost on real hardware).

phi everywhere uses z = y + b + 1 accumulated directly in PSUM (the
conv bias + 1 enters as a rank-1 matmul appended to each accumulation
group), giving the 2-op form  max(z, min(exp(z-1), 1)):
one ACT exp (bias=-1) + one DVE scalar_tensor_tensor.
Matmuls run in bf16 (f32 PSUM accumulate) for pipelined LDWEIGHTS.
Phases run NT -> KV -> Q -> OUT so the ACT table switches Exp->Gelu
exactly once. Residual add and the kv PSUM->SBUF copy run on DVE
(Pool's TensorTensor is ~1.1us/tile and would serialize the OUT tail;
ACT Copy would thrash the activation-table set).
"""

import ml_dtypes
import numpy as np

import concourse.bass as bass
import concourse.mybir as mybir
import concourse.tile as tile
from concourse import bacc
from concourse.bass_utils import run_bass_kernel_spmd

F32 = mybir.dt.float32
BF16 = mybir.dt.bfloat16
AF = mybir.ActivationFunctionType
ALU = mybir.AluOpType

B, C, N = 8, 256, 4096
NCORES = 8
CT = C // 128        # 2 c-tiles (partition groups) per 256-channel dim
NJ = N // 512        # 8 column chunks of 512
NT = N // 128        # 32 n-tiles of 128
NP = N + 2           # x padded with one zero column on each side
HEAD = 1026          # first two j-chunks (+2 pad cols) of xb, DMA'd first
KW = 3 * CT * 256    # one wt half: 6 blocks of [128, 256]

BF = ml_dtypes.bfloat16


def _build_nc():
    nc = bacc.Bacc("TRN2", target_bir_lowering=False, debug=False, num_devices=NCORES)

    xb_d = nc.declare_dram_parameter("xb", [CT, 128, NP], BF16, isOutput=False)
    vt_d = nc.declare_dram_parameter("vt", [128, NT * 256], BF16, isOutput=False)
    wt_d = nc.declare_dram_parameter("wt", [128, 2 * KW], BF16, isOutput=False)
    neg1_d = nc.declare_dram_parameter("neg1", [128, 1], F32, isOutput=False)
    row_d = nc.declare_dram_parameter("row", [1, 1024], BF16, isOutput=False)
    out_d = nc.declare_dram_parameter("out", [C, N], BF16, isOutput=True)

    with tile.TileContext(nc) as tc:
        with (
            tc.tile_pool(name="persist", bufs=1) as per,
            tc.tile_pool(name="tmp", bufs=6) as tmp,
            tc.tile_pool(name="psum", bufs=6, space="PSUM") as ps,
            tc.tile_pool(name="psum2", bufs=2, space="PSUM") as ps2,
        ):
            # ---- inputs: 11 large DMAs, start-critical ones first --------
            # (xb in three n-slices per ci so the first conv tiles can
            # start while the bulk is still in flight)
            wt_sb = per.tile([128, 2 * KW], BF16, tag="wt")
            nc.sync.dma_start(out=wt_sb[:, 0:KW], in_=wt_d[:, 0:KW])
            xb_sb = [per.tile([128, NP], BF16, tag=f"xb{ci}", name=f"xb{ci}")
                     for ci in range(CT)]
            for ci in range(CT):
                nc.sync.dma_start(out=xb_sb[ci][:, 0:514],
                                  in_=xb_d[ci, :, 0:514])
            row_sb = per.tile([1, 1024], BF16, tag="row")
            nc.sync.dma_start(out=row_sb, in_=row_d[:, :])
            neg1_sb = per.tile([128, 1], F32, tag="neg1")
            nc.sync.dma_start(out=neg1_sb, in_=neg1_d[:, :])
            for ci in range(CT):
                nc.sync.dma_start(out=xb_sb[ci][:, 514:2050],
                                  in_=xb_d[ci, :, 514:2050])
            for ci in range(CT):
                nc.sync.dma_start(out=xb_sb[ci][:, 2050:NP],
                                  in_=xb_d[ci, :, 2050:NP])
            vt_sb = per.tile([128, NT * 256], BF16, tag="vt")
            nc.sync.dma_start(out=vt_sb, in_=vt_d[:, :])
            nc.sync.dma_start(out=wt_sb[:, KW:2 * KW], in_=wt_d[:, KW:2 * KW])

            ones512 = row_sb[0:1, 0:512]       # Q bias matmul moving operand
            onesk = row_sb[0:1, 0:128]         # NT bias matmul stationary
            bk1 = row_sb[0:1, 512:768]         # conv_b[k half] + 1

            def wk(t, cit):                    # k-half weights [128(ci), 256(co)]
                o = (t * CT + cit) * 256
                return wt_sb[:, o:o + 256]

            def wq(t, cit):                    # q-half weights [128(ci), 256(co)]
                o = KW + (t * CT + cit) * 256
                return wt_sb[:, o:o + 256]

            # ---- persistent intermediates --------------------------------
            kT = per.tile([128, NT, 256], BF16, tag="kT")    # phi(k) in [n, c]
            qphi = [per.tile([128, N], BF16, tag=f"qphi{ct}", name=f"qphi{ct}")
                    for ct in range(CT)]
            kv_sb = per.tile([128, CT, 256], BF16, tag="kv")  # kv in [c, d]

            # ---- phase NT: k^T = phi(conv_k + b) in [n, c] layout --------
            for i in range(NT):
                j, off = i // 4, (i % 4) * 128
                kt_ps = ps.tile([128, 512], F32, tag="bank", name="kt_ps")
                kt_ps = kt_ps[:, 0:256]
                for ci in range(CT):
                    for t in range(3):
                        nc.tensor.matmul(
                            kt_ps,
                            xb_sb[ci][:, j * 512 + off + t:
                                      j * 512 + off + t + 128],
                            wk(t, ci),
                            start=(ci == 0 and t == 0),
                            stop=False,
                        )
                # z = y + (b_k + 1): rank-1 (ones^T @ bk1) ends the group
                nc.tensor.matmul(kt_ps, onesk, bk1, start=False, stop=True)
                # phi = max(z, min(exp(z-1), 1))
                e = tmp.tile([128, 256], F32, tag="nte")
                nc.scalar.activation(e, kt_ps, AF.Exp, bias=neg1_sb)
                nc.vector.scalar_tensor_tensor(
                    kT[:, i, :], e, 1.0, kt_ps, ALU.min, ALU.max)

            # ---- phase Q: q = phi(conv_q + b) in [c, n] layout -----------
            for ct in range(CT):
                bq1 = row_sb[0:1, 768 + ct * 128:768 + (ct + 1) * 128]
                for j in range(NJ):
                    q_ps = ps.tile([128, 512], F32, tag="bank", name="q_ps")
                    for ci in range(CT):
                        for t in range(3):
                            nc.tensor.matmul(
                                q_ps,
                                wq(t, ci)[:, ct * 128:(ct + 1) * 128],
                                xb_sb[ci][:, j * 512 + t:j * 512 + t + 512],
                                start=(ci == 0 and t == 0),
                                stop=False,
                            )
                    # z = y + (b_q + 1): rank-1 (bq1^T @ ones) ends the group
                    nc.tensor.matmul(q_ps, bq1, ones512, start=False, stop=True)
                    e = tmp.tile([128, 512], F32, tag="qte")
                    nc.scalar.activation(e, q_ps, AF.Exp, bias=neg1_sb)
                    nc.vector.scalar_tensor_tensor(
                        qphi[ct][:, j * 512:(j + 1) * 512],
                        e, 1.0, q_ps, ALU.min, ALU.max)

            # ---- phase KV: kv[c, d] = sum_n k^T[n, c] v^T[n, d] ----------
            for ch in range(CT):
                kv_ps = ps2.tile([128, 512], F32, tag="kvp", name="kv_ps")
                kv_ps = kv_ps[:, 0:256]
                for i in range(NT):
                    nc.tensor.matmul(
                        kv_ps,
                        kT[:, i, ch * 128:(ch + 1) * 128],
                        vt_sb[:, i * 256:(i + 1) * 256],
                        start=(i == 0),
                        stop=(i == NT - 1),
                    )
                nc.vector.tensor_copy(kv_sb[:, ch, :], kv_ps)

            # ---- phase OUT: out[d, n] = gelu(sum_c kv[c, d] q[c, n]) + x -
            # pairs of j-chunks share one residual add + one output DMA so
            # the HWDGE issue rate (625ns/DMA) stays ahead of ACT's gelu
            # rate and the tail drains fast
            for dt in range(CT):
                for jj in range(NJ // 2):
                    g = tmp.tile([128, 1024], BF16, tag="og")
                    for h in range(2):
                        j = 2 * jj + h
                        o_ps = ps.tile([128, 512], F32, tag="bank",
                                       name="o_ps")
                        for ch in range(CT):
                            nc.tensor.matmul(
                                o_ps,
                                kv_sb[:, ch, dt * 128:(dt + 1) * 128],
                                qphi[ch][:, j * 512:(j + 1) * 512],
                                start=(ch == 0),
                                stop=(ch == CT - 1),
                            )
                        nc.scalar.activation(
                            g[:, h * 512:(h + 1) * 512], o_ps, AF.Gelu)
                    o = tmp.tile([128, 1024], BF16, tag="oo")
                    nc.vector.tensor_add(
                        o, g, xb_sb[dt][:, 1 + jj * 1024:1 + (jj + 1) * 1024])
                    nc.sync.dma_start(
                        out=out_d[dt * 128:(dt + 1) * 128,
                                  jj * 1024:(jj + 1) * 1024],
                        in_=o,
                    )

    nc.compile()
    return nc


_NC_CACHE = None


def _get_nc():
    global _NC_CACHE
    if _NC_CACHE is None:
        _NC_CACHE = _build_nc()
    return _NC_CACHE


def _prep(x, conv_w, conv_b):
    x = np.asarray(x, dtype=np.float32)
    conv_w = np.asarray(conv_w, dtype=np.float32)
    conv_b = np.asarray(conv_b, dtype=np.float32)
    xb = np.zeros((B, CT, 128, NP), dtype=BF)
    xb[:, :, :, 1:N + 1] = x.reshape(B, CT, 128, N).astype(BF)
    # vt[b, p, i*256 + d] = x[b, d, i*128 + p]
    xt = x.transpose(0, 2, 1)                              # [B, N, C]
    vt = np.ascontiguousarray(
        xt.reshape(B, NT, 128, C).transpose(0, 2, 1, 3)
    ).reshape(B, 128, NT * C).astype(BF)
    # wt[ci, half, (t*CT + cit)*256 + co'] = conv_w[half*256 + co', cit*128 + ci, t]
    w4 = (conv_w.transpose(1, 2, 0)                        # [cin, t, co]
          .reshape(CT, 128, 3, 2 * C)                      # [cit, ci, t, co]
          .transpose(1, 2, 0, 3))                          # [ci, t, cit, co]
    wt = np.concatenate(
        [w4[..., C:2 * C].reshape(128, KW),                # k half first
         w4[..., 0:C].reshape(128, KW)],
        axis=1).astype(BF)
    neg1 = np.full((128, 1), -1.0, dtype=np.float32)
    row = np.zeros((1, 1024), dtype=np.float32)
    row[0, 0:512] = 1.0
    row[0, 512:768] = conv_b[C:] + 1.0
    row[0, 768:1024] = conv_b[0:C] + 1.0
    return xb, vt, wt, neg1, row.astype(BF)


def make_in_maps(x, conv_w, conv_b):
    xb, vt, wt, neg1, row = _prep(x, conv_w, conv_b)
    return [
        {"xb": xb[b], "vt": vt[b], "wt": wt, "neg1": neg1, "row": row}
        for b in range(B)
    ]


def kernel(x: np.ndarray, conv_w: np.ndarray, conv_b: np.ndarray) -> np.ndarray:
    nc = _get_nc()
    in_maps = make_in_maps(x, conv_w, conv_b)
    res = run_bass_kernel_spmd(nc, in_maps, core_ids=list(range(NCORES)))
    return np.stack(
        [res.results[b]["out"].astype(np.float32) for b in range(B)], axis=0)


# revision 18
# speedup vs baseline: 1.3743x; 1.1005x over previous
"""Trainium2 Bass kernel for nn_AttentionLayer (conv1d -> linear attention -> gelu + residual).

Full inputs:  x [8, 256, 4096] f32, conv_w [512, 256, 3] f32, conv_b [512] f32
Full output:  [8, 256, 4096] f32

Sharding: pure data-parallel over batch B=8 -> 8 NeuronCores, one batch each.
No collectives needed.

Per-core math (C=256, N=4096, one batch):
  y    = conv1d(x, w, pad=1) + b          # [2C, N]
  q    = phi(y[:C]),  k = phi(y[C:])      # phi = elu+1
  v    = x^T                              # [N, C]
  kv   = sum_n phi(k)[n,:] (x) v[n,:]     # [C, C]
  out  = gelu(q @ kv) + x                 # [C, N]

Layout trick: the conv contraction (over input channels ci) produces
q in [c, n] layout (w^T stationary) AND k in [n, c] layout (x stationary)
with zero transposes; all other operands are host-prepped into layouts
that give one large contiguous-per-partition DMA per tensor (10 input
DMAs total — the HWDGE issue path serializes at ~650ns/DMA, so DMA
count is a first-order cost on real hardware).

phi everywhere uses z = y + b + 1 accumulated directly in PSUM (the
conv bias + 1 enters as a rank-1 matmul appended to each accumulation
group), giving the 2-op form  max(z, min(exp(z-1), 1)):
one ACT exp (bias=-1) + one DVE scalar_tensor_tensor.
Matmuls run in bf16 (f32 PSUM accumulate) for pipelined LDWEIGHTS.
Phases run NT -> KV -> Q -> OUT so the ACT table switches Exp->Gelu
exactly once. Residual add and the kv PSUM->SBUF copy run on DVE
(Pool's TensorTensor is ~1.1us/tile and would serialize the OUT tail;
ACT Copy would thrash the activation-table set).
"""

import ml_dtypes
import numpy as np

import concourse.bass as bass
import concourse.mybir as mybir
import concourse.tile as tile
from concourse import bacc
from concourse.bass_utils import run_bass_kernel_spmd

F32 = mybir.dt.float32
BF16 = mybir.dt.bfloat16
AF = mybir.ActivationFunctionType
ALU = mybir.AluOpType

B, C, N = 8, 256, 4096
NCORES = 8
CT = C // 128        # 2 c-tiles (partition groups) per 256-channel dim
NJ = N // 512        # 8 column chunks of 512
NT = N // 128        # 32 n-tiles of 128
NP = N + 2           # x padded with one zero column on each side
HEAD = 1026          # first two j-chunks (+2 pad cols) of xb, DMA'd first
KW = 3 * CT * 256    # one wt half: 6 blocks of [128, 256]

BF = ml_dtypes.bfloat16


def _build_nc():
    nc = bacc.Bacc("TRN2", target_bir_lowering=False, debug=False, num_devices=NCORES)

    xb_d = nc.declare_dram_parameter("xb", [CT, 128, NP], BF16, isOutput=False)
    vt_d = nc.declare_dram_parameter("vt", [128, NT * 256], BF16, isOutput=False)
    wt_d = nc.declare_dram_parameter("wt", [128, 2 * KW], BF16, isOutput=False)
    neg1_d = nc.declare_dram_parameter("neg1", [128, 1], F32, isOutput=False)
    row_d = nc.declare_dram_parameter("row", [1, 1024], BF16, isOutput=False)
    out_d = nc.declare_dram_parameter("out", [C, N], BF16, isOutput=True)

    with tile.TileContext(nc) as tc:
        with (
            tc.tile_pool(name="persist", bufs=1) as per,
            tc.tile_pool(name="tmp", bufs=6) as tmp,
            tc.tile_pool(name="psum", bufs=4, space="PSUM") as ps,
        ):
            # ---- inputs: 11 large DMAs, start-critical ones first --------
            # (xb in three n-slices per ci so the first conv tiles can
            # start while the bulk is still in flight)
            wt_sb = per.tile([128, 2 * KW], BF16, tag="wt")
            xb_sb = [per.tile([128, NP], BF16, tag=f"xb{ci}", name=f"xb{ci}")
                     for ci in range(CT)]
            for ci in range(CT):
                nc.sync.dma_start(out=wt_sb[:, ci * 768:(ci + 1) * 768],
                                  in_=wt_d[:, ci * 768:(ci + 1) * 768])
                nc.sync.dma_start(out=xb_sb[ci][:, 0:514],
                                  in_=xb_d[ci, :, 0:514])
            row_sb = per.tile([1, 1024], BF16, tag="row")
            nc.sync.dma_start(out=row_sb, in_=row_d[:, :])
            neg1_sb = per.tile([128, 1], F32, tag="neg1")
            nc.sync.dma_start(out=neg1_sb, in_=neg1_d[:, :])
            for ci in range(CT):
                nc.sync.dma_start(out=xb_sb[ci][:, 514:2050],
                                  in_=xb_d[ci, :, 514:2050])
            for ci in range(CT):
                nc.sync.dma_start(out=xb_sb[ci][:, 2050:NP],
                                  in_=xb_d[ci, :, 2050:NP])
            vt_sb = per.tile([128, NT * 256], BF16, tag="vt")
            nc.sync.dma_start(out=vt_sb, in_=vt_d[:, :])
            nc.sync.dma_start(out=wt_sb[:, KW:2 * KW], in_=wt_d[:, KW:2 * KW])

            ones512 = row_sb[0:1, 0:512]       # Q bias matmul moving operand
            onesk = row_sb[0:1, 0:128]         # NT bias matmul stationary
            bk1 = row_sb[0:1, 512:768]         # conv_b[k half] + 1

            def wk(t, cit):                    # k-half weights [128(ci), 256(co)]
                o = (cit * 3 + t) * 256
                return wt_sb[:, o:o + 256]

            def wq(t, cit):                    # q-half weights [128(ci), 256(co)]
                o = KW + (t * CT + cit) * 256
                return wt_sb[:, o:o + 256]

            # ---- persistent intermediates --------------------------------
            kT = per.tile([128, NT, 256], BF16, tag="kT")    # phi(k) in [n, c]
            qphi = [per.tile([128, N], BF16, tag=f"qphi{ct}", name=f"qphi{ct}")
                    for ct in range(CT)]
            kv_sb = per.tile([128, CT, 256], BF16, tag="kv")  # kv in [c, d]

            # ---- phase NT: k^T = phi(conv_k + b) in [n, c] layout --------
            for i in range(NT):
                j, off = i // 4, (i % 4) * 128
                kt_ps = ps.tile([128, 1024], F32, tag="bank", name="kt_ps")
                kt_ps = kt_ps[:, 0:256]
                for ci in range(CT):
                    for t in range(3):
                        nc.tensor.matmul(
                            kt_ps,
                            xb_sb[ci][:, j * 512 + off + t:
                                      j * 512 + off + t + 128],
                            wk(t, ci),
                            start=(ci == 0 and t == 0),
                            stop=False,
                        )
                # z = y + (b_k + 1): rank-1 (ones^T @ bk1) ends the group
                nc.tensor.matmul(kt_ps, onesk, bk1, start=False, stop=True)
                # phi = max(z, min(exp(z-1), 1))
                e = tmp.tile([128, 256], F32, tag="nte")
                nc.scalar.activation(e, kt_ps, AF.Exp, bias=neg1_sb)
                nc.vector.scalar_tensor_tensor(
                    kT[:, i, :], e, 1.0, kt_ps, ALU.min, ALU.max)

            # ---- phase Q: q = phi(conv_q + b) in [c, n] layout -----------
            for ct in range(CT):
                bq1 = row_sb[0:1, 768 + ct * 128:768 + (ct + 1) * 128]
                for j in range(NJ):
                    q_ps = ps.tile([128, 1024], F32, tag="bank",
                                   name="q_ps")
                    q_ps = q_ps[:, 0:512]
                    for ci in range(CT):
                        for t in range(3):
                            nc.tensor.matmul(
                                q_ps,
                                wq(t, ci)[:, ct * 128:(ct + 1) * 128],
                                xb_sb[ci][:, j * 512 + t:j * 512 + t + 512],
                                start=(ci == 0 and t == 0),
                                stop=False,
                            )
                    # z = y + (b_q + 1): rank-1 (bq1^T @ ones) ends the group
                    nc.tensor.matmul(q_ps, bq1, ones512, start=False, stop=True)
                    e = tmp.tile([128, 512], F32, tag="qte")
                    nc.scalar.activation(e, q_ps, AF.Exp, bias=neg1_sb)
                    nc.vector.scalar_tensor_tensor(
                        qphi[ct][:, j * 512:(j + 1) * 512],
                        e, 1.0, q_ps, ALU.min, ALU.max)

            # ---- phase KV: kv[c, d] = sum_n k^T[n, c] v^T[n, d] ----------
            for ch in range(CT):
                kv_ps = ps.tile([128, 1024], F32, tag="bank", name="kv_ps")
                kv_ps = kv_ps[:, 0:256]
                for i in range(NT):
                    nc.tensor.matmul(
                        kv_ps,
                        kT[:, i, ch * 128:(ch + 1) * 128],
                        vt_sb[:, i * 256:(i + 1) * 256],
                        start=(i == 0),
                        stop=(i == NT - 1),
                    )
                nc.vector.tensor_copy(kv_sb[:, ch, :], kv_ps)

            # ---- phase OUT: out[d, n] = gelu(sum_c kv[c, d] q[c, n]) + x -
            # pairs of j-chunks share one residual add + one output DMA so
            # the HWDGE issue rate (625ns/DMA) stays ahead of ACT's gelu
            # rate and the tail drains fast
            for dt in range(CT):
                for jj in range(NJ // 2):
                    o_ps = ps.tile([128, 1024], F32, tag="bank",
                                   name="o_ps")
                    for h in range(2):
                        j = 2 * jj + h
                        for ch in range(CT):
                            nc.tensor.matmul(
                                o_ps[:, h * 512:(h + 1) * 512],
                                kv_sb[:, ch, dt * 128:(dt + 1) * 128],
                                qphi[ch][:, j * 512:(j + 1) * 512],
                                start=(ch == 0),
                                stop=(ch == CT - 1),
                            )
                    g = tmp.tile([128, 1024], BF16, tag="og")
                    nc.scalar.activation(g, o_ps, AF.Gelu)
                    o = tmp.tile([128, 1024], BF16, tag="oo")
                    nc.vector.tensor_add(
                        o, g, xb_sb[dt][:, 1 + jj * 1024:1 + (jj + 1) * 1024])
                    nc.sync.dma_start(
                        out=out_d[dt * 128:(dt + 1) * 128,
                                  jj * 1024:(jj + 1) * 1024],
                        in_=o,
                    )

    nc.compile()
    return nc


_NC_CACHE = None


def _get_nc():
    global _NC_CACHE
    if _NC_CACHE is None:
        _NC_CACHE = _build_nc()
    return _NC_CACHE


def _prep(x, conv_w, conv_b):
    x = np.asarray(x, dtype=np.float32)
    conv_w = np.asarray(conv_w, dtype=np.float32)
    conv_b = np.asarray(conv_b, dtype=np.float32)
    xb = np.zeros((B, CT, 128, NP), dtype=BF)
    xb[:, :, :, 1:N + 1] = x.reshape(B, CT, 128, N).astype(BF)
    # vt[b, p, i*256 + d] = x[b, d, i*128 + p]
    xt = x.transpose(0, 2, 1)                              # [B, N, C]
    vt = np.ascontiguousarray(
        xt.reshape(B, NT, 128, C).transpose(0, 2, 1, 3)
    ).reshape(B, 128, NT * C).astype(BF)
    # wt[ci, half, (t*CT + cit)*256 + co'] = conv_w[half*256 + co', cit*128 + ci, t]
    w4 = (conv_w.transpose(1, 2, 0)                        # [cin, t, co]
          .reshape(CT, 128, 3, 2 * C)                      # [cit, ci, t, co]
          .transpose(1, 2, 0, 3))                          # [ci, t, cit, co]
    wt = np.concatenate(
        [w4[..., C:2 * C].transpose(0, 2, 1, 3)            # k half, cit-major
         .reshape(128, KW),
         w4[..., 0:C].reshape(128, KW)],                   # q half, t-major
        axis=1).astype(BF)
    neg1 = np.full((128, 1), -1.0, dtype=np.float32)
    row = np.zeros((1, 1024), dtype=np.float32)
    row[0, 0:512] = 1.0
    row[0, 512:768] = conv_b[C:] + 1.0
    row[0, 768:1024] = conv_b[0:C] + 1.0
    return xb, vt, wt, neg1, row.astype(BF)


def make_in_maps(x, conv_w, conv_b):
    xb, vt, wt, neg1, row = _prep(x, conv_w, conv_b)
    return [
        {"xb": xb[b], "vt": vt[b], "wt": wt, "neg1": neg1, "row": row}
        for b in range(B)
    ]


def kernel(x: np.ndarray, conv_w: np.ndarray, conv_b: np.ndarray) -> np.ndarray:
    nc = _get_nc()
    in_maps = make_in_maps(x, conv_w, conv_b)
    res = run_bass_kernel_spmd(nc, in_maps, core_ids=list(range(NCORES)))
    return np.stack(
        [res.results[b]["out"].astype(np.float32) for b in range(B)], axis=0)
